# revision 6
# baseline (speedup 1.0000x reference)
"""Bag-of-words histogram kernel for Trainium2 (8 NeuronCores, data-parallel).

Problem: inputs [1024, 512] int32 token ids in [0, 50257); output [1024, 50256]
f32 per-row token-count histogram with token id 0 dropped.

Design (per core, 128 rows):
  Shifted token u = t - 1 decomposes as u = hi*394 + lo with hi in [-1,128),
  lo in [0,394) (exact integer div via multiply-arith-shift: hi =
  (u*21291)>>23, verified offline for all u in [-1, 50256)). Token t=0 gives
  hi=-1 which matches no hi-bin, so the "drop token 0" semantics falls out of
  the decomposition for free and output bin v = u = hi*394 + lo directly.

  Per row the histogram is hist[hi, lo] = sum_j onehot_hi(u_j)[hi] *
  onehot_lo(u_j)[lo]: a matmul with lhsT = A [K=128 tokens, M=128 hi-bins]
  f16 one-hot, rhs = B [K=128 tokens, N=394 lo-bins] f16 one-hot, accumulated
  over 4 K-chunks of 128 tokens into one PSUM bank [128, 394] f32. Duplicate
  tokens are handled exactly by the accumulation.

  Engine split (from CoreSim engine-busy analysis):
    - DVE builds all wide B one-hots (f16, 4x perf mode).
    - GPSIMD (otherwise idle) builds all A one-hots.
    - ACT only evacuates PSUM -> SBUF f32, packing GR=8 rows per wide tile.
    - SP issues the output DMAs. Every HWDGE dma_start costs ~625ns of the
      shared descriptor generator regardless of size, so DMAs are batched:
      one [127-partition x 8-row x 1576B-line] piece (v = 0..50037 for 8
      rows) plus one batched tail piece (v = 50038..50255, partition 127)
      per 8 rows -> 32 output DMAs total instead of 384.

  One-hots are built with tensor_scalar(is_equal) against constant iota rows,
  using per-partition f32 scalars = the transposed hi/lo digit of each token
  (DMA-transposed i16 digit tensors).
"""

import sys

sys.path.insert(0, "/opt/trn_rl_repo")

import numpy as np

N_CORES = 8
B_FULL = 1024
P = 128  # rows per core / partitions
S = 512  # tokens per row
V = 50257
R1 = 128  # hi bins (partition dim of PSUM)
R2 = 394  # lo bins (free dim of PSUM)
DIV_MUL = 21291  # (u*DIV_MUL)>>23 (arith) == u//394 for all u in [-1, 50256)
DIV_SH = 23
NCH = S // P  # 4 K-chunks per row
OUT_COLS = V - 1  # 50256

BIG = 127 * R2  # 50038: v = 0..50037 from partitions 0..126
TAIL = OUT_COLS - BIG  # 218: v = 50038..50255 from partition 127 cols 0..217

POOL_CHUNKS = (0, 1, 2, 3)  # a_t one-hots built on GPSIMD for these chunks
GR = 8  # rows batched per output DMA

_CACHED = {}


def _emit(nc, tc, tile, mybir, tok_dram, out_dram, dyn_reps=None):
    alu = mybir.AluOpType

    with (
        tc.tile_pool(name="const", bufs=1) as const,
        tc.tile_pool(name="prep", bufs=1) as prep,
        tc.tile_pool(name="oh_a", bufs=24) as oh_a,
        tc.tile_pool(name="oh_b", bufs=24) as oh_b,
        tc.tile_pool(name="evac", bufs=3) as evac,
        tc.tile_pool(name="psum", bufs=8, space="PSUM") as psum,
    ):
        # constant iota rows (same in every partition)
        iota_i16 = const.tile([P, R2], mybir.dt.int16)
        nc.gpsimd.iota(iota_i16[:], pattern=[[1, R2]], base=0, channel_multiplier=0)
        iota_lo = const.tile([P, R2], mybir.dt.float16)
        nc.vector.tensor_copy(iota_lo[:], iota_i16[:])
        iota_hi = const.tile([P, R1], mybir.dt.float16)
        nc.vector.tensor_copy(iota_hi[:], iota_i16[:, :R1])

        # load tokens, shift by -1, split digits
        tok = prep.tile([P, S], mybir.dt.int32)
        nc.sync.dma_start(tok[:], tok_dram[:])
        u32 = prep.tile([P, S], mybir.dt.int32)
        nc.vector.tensor_scalar(u32[:], tok[:], 1, None, op0=alu.subtract)
        hprod = prep.tile([P, S], mybir.dt.int32)
        nc.vector.tensor_scalar(hprod[:], u32[:], DIV_MUL, None, op0=alu.mult)
        hi32 = prep.tile([P, S], mybir.dt.int32)
        nc.vector.tensor_scalar(
            hi32[:], hprod[:], DIV_SH, None, op0=alu.arith_shift_right
        )
        him = prep.tile([P, S], mybir.dt.int32)
        nc.vector.tensor_scalar(him[:], hi32[:], R2, None, op0=alu.mult)
        lo32 = prep.tile([P, S], mybir.dt.int32)
        nc.vector.tensor_tensor(lo32[:], u32[:], him[:], op=alu.subtract)

        hi16 = prep.tile([P, S], mybir.dt.int16)
        nc.vector.tensor_copy(hi16[:], hi32[:])
        lo16 = prep.tile([P, S], mybir.dt.int16)
        nc.vector.tensor_copy(lo16[:], lo32[:])

        # transpose each 128-col chunk: digT[p, c*128+b] = dig[b, c*128+p]
        hiT16 = prep.tile([P, S], mybir.dt.int16)
        loT16 = prep.tile([P, S], mybir.dt.int16)
        for c in range(NCH):
            sl = slice(c * P, (c + 1) * P)
            nc.sync.dma_start_transpose(hiT16[:, sl], hi16[:, sl])
            nc.sync.dma_start_transpose(loT16[:, sl], lo16[:, sl])

        # f32 per-partition scalar sources
        hiT = prep.tile([P, S], mybir.dt.float32)
        nc.vector.tensor_copy(hiT[:], hiT16[:])
        loT = prep.tile([P, S], mybir.dt.float32)
        nc.vector.tensor_copy(loT[:], loT16[:])

        def block_body(b0):
            # ev packs GR rows: ev[p, g*R2 + lo] = hist(row b0+g)[p*R2 + lo]
            ev = evac.tile([P, GR * R2], mybir.dt.float32)
            for g in range(GR):
                b = b0 + g
                ps = psum.tile([P, R2], mybir.dt.float32)
                for c in range(NCH):
                    col = c * P + b
                    b_t = oh_b.tile([P, R2], mybir.dt.float16)
                    nc.vector.tensor_scalar(
                        b_t[:], iota_lo[:], loT[:, col : col + 1], None,
                        op0=alu.is_equal,
                    )
                    a_t = oh_a.tile([P, R1], mybir.dt.float16)
                    eng = nc.gpsimd if c in POOL_CHUNKS else nc.vector
                    eng.tensor_scalar(
                        a_t[:], iota_hi[:], hiT[:, col : col + 1], None,
                        op0=alu.is_equal,
                    )
                    nc.tensor.matmul(
                        ps[:], a_t[:], b_t[:], start=(c == 0), stop=(c == NCH - 1)
                    )
                # DMA cannot read PSUM; evacuate via otherwise-idle ScalarE
                nc.scalar.copy(ev[:, g * R2 : (g + 1) * R2], ps[:])
            # v = 0..50037 for all GR rows in one DMA: DRAM traversal
            # (p, g, lo) at address (b0+g)*OUT_COLS + p*R2 + lo matches the
            # SBUF traversal (partition p, then free (g, lo)).
            big_dst = out_dram[b0 : b0 + GR, 0:BIG].rearrange(
                "g (p l) -> p g l", p=127, l=R2
            )
            nc.sync.dma_start(big_dst, ev[0:127, :])
            # v = 50038..50255 (partition 127, first TAIL cols of each row)
            tail_src = ev[127:128, :].rearrange("p (g l) -> p g l", g=GR, l=R2)[
                :, :, 0:TAIL
            ]
            nc.sync.dma_start(out_dram[b0 : b0 + GR, BIG:OUT_COLS], tail_src)

        if dyn_reps is None:
            for b0 in range(0, P, GR):
                block_body(b0)
        else:
            with tc.For_i(0, dyn_reps, 1):
                for b0 in range(0, P, GR):
                    block_body(b0)


def _build_program():
    import concourse.tile as tile
    from concourse import bacc, mybir

    nc = bacc.Bacc(
        "TRN2",
        target_bir_lowering=False,
        debug=False,
        enable_asserts=False,
        num_devices=N_CORES,
    )
    tok_dram = nc.dram_tensor("inputs", [P, S], mybir.dt.int32, kind="ExternalInput").ap()
    out_dram = nc.dram_tensor(
        "out", [P, OUT_COLS], mybir.dt.float32, kind="ExternalOutput"
    ).ap()

    with tile.TileContext(nc) as tc:
        _emit(nc, tc, tile, mybir, tok_dram, out_dram)

    nc.compile()
    return nc


def _build_program_dyn():
    """Variant with a runtime repeat loop around the row loop, for HW timing.

    Trip count comes from the extra [1,1] uint32 input "reps" — same NEFF for
    any R, so wall-time slope over R isolates device execution time.
    """
    import concourse.tile as tile
    from concourse import bacc, mybir

    nc = bacc.Bacc(
        "TRN2",
        target_bir_lowering=False,
        debug=False,
        enable_asserts=False,
        num_devices=N_CORES,
    )
    tok_dram = nc.dram_tensor("inputs", [P, S], mybir.dt.int32, kind="ExternalInput").ap()
    reps_dram = nc.dram_tensor("reps", [1, 1], mybir.dt.uint32, kind="ExternalInput").ap()
    out_dram = nc.dram_tensor(
        "out", [P, OUT_COLS], mybir.dt.float32, kind="ExternalOutput"
    ).ap()

    with tile.TileContext(nc) as tc:
        with tc.tile_pool(name="repsld", bufs=1) as repsld:
            reps_sb = repsld.tile([1, 1], mybir.dt.uint32)
            nc.sync.dma_start(reps_sb[:], reps_dram[:])

            from concourse.bass_primitives_rust import RegisterHandles
            from concourse.expressions import make_scalar_value

            regs = []
            for eng in (nc.sync, nc.vector, nc.scalar, nc.tensor, nc.gpsimd):
                tmp = eng.alloc_register(f"reps_{eng.engine.value}")
                eng.reg_load(tmp, reps_sb[0:1, 0:1])
                regs.append(tmp)
            rv = make_scalar_value(RegisterHandles(regs), min_val=0, max_val=1 << 20)

            _emit(nc, tc, tile, mybir, tok_dram, out_dram, dyn_reps=rv)

    nc.compile()
    return nc


def _get_program():
    if "nc" not in _CACHED:
        _CACHED["nc"] = _build_program()
    return _CACHED["nc"]


def kernel(inputs: np.ndarray, _trace: bool = False, _tmpdir: str | None = None):
    from concourse.bass_utils import run_bass_kernel_spmd

    nc = _get_program()
    inputs = np.ascontiguousarray(np.asarray(inputs, dtype=np.int32))
    assert inputs.shape == (B_FULL, S), inputs.shape
    in_maps = [{"inputs": inputs[k * P : (k + 1) * P]} for k in range(N_CORES)]
    res = run_bass_kernel_spmd(
        nc,
        in_maps,
        core_ids=list(range(N_CORES)),
        trace=_trace,
        tmpdir=_tmpdir,
    )
    out = np.concatenate([r["out"] for r in res.results], axis=0)
    if _trace:
        _CACHED["last_results"] = res
    return out


# revision 7
# speedup vs baseline: 1.1359x; 1.1359x over previous
"""Bag-of-words histogram kernel for Trainium2 (8 NeuronCores, data-parallel).

Problem: inputs [1024, 512] int32 token ids in [0, 50257); output [1024, 50256]
f32 per-row token-count histogram with token id 0 dropped.

Design (per core, 128 rows):
  Shifted token u = t - 1 decomposes as u = hi*394 + lo with hi in [-1,128),
  lo in [0,394) (exact integer div via multiply-arith-shift: hi =
  (u*21291)>>23, verified offline for all u in [-1, 50256)). Token t=0 gives
  hi=-1 which matches no hi-bin, so the "drop token 0" semantics falls out of
  the decomposition for free and output bin v = u = hi*394 + lo directly.

  Per row the histogram is hist[hi, lo] = sum_j onehot_hi(u_j)[hi] *
  onehot_lo(u_j)[lo]: a matmul with lhsT = A [K=128 tokens, M=128 hi-bins]
  f16 one-hot, rhs = B [K=128 tokens, N=394 lo-bins] f16 one-hot, accumulated
  over 4 K-chunks of 128 tokens into one PSUM bank [128, 394] f32. Duplicate
  tokens are handled exactly by the accumulation.

  Engine split (from CoreSim engine-busy analysis):
    - DVE builds all wide B one-hots (f16, 4x perf mode).
    - GPSIMD (otherwise idle) builds all A one-hots.
    - ACT only evacuates PSUM -> SBUF f32, packing GR=8 rows per wide tile.
    - SP issues the output DMAs. Every HWDGE dma_start costs ~625ns of the
      shared descriptor generator regardless of size, so DMAs are batched:
      one [127-partition x 8-row x 1576B-line] piece (v = 0..50037 for 8
      rows) plus one batched tail piece (v = 50038..50255, partition 127)
      per 8 rows -> 32 output DMAs total instead of 384.

  One-hots are built with tensor_scalar(is_equal) against constant iota rows,
  using per-partition f32 scalars = the transposed hi/lo digit of each token
  (DMA-transposed i16 digit tensors).
"""

import sys

sys.path.insert(0, "/opt/trn_rl_repo")

import numpy as np

N_CORES = 8
B_FULL = 1024
P = 128  # rows per core / partitions
S = 512  # tokens per row
V = 50257
R1 = 128  # hi bins (partition dim of PSUM)
R2 = 394  # lo bins (free dim of PSUM)
DIV_MUL = 21291  # (u*DIV_MUL)>>23 (arith) == u//394 for all u in [-1, 50256)
DIV_SH = 23
NCH = S // P  # 4 K-chunks per row
OUT_COLS = V - 1  # 50256

BIG = 127 * R2  # 50038: v = 0..50037 from partitions 0..126
TAIL = OUT_COLS - BIG  # 218: v = 50038..50255 from partition 127 cols 0..217

POOL_CHUNKS = ()  # a_t one-hots built on GPSIMD for these chunks (HW: GPSIMD
# tensor ops cost ~2us launch overhead each, far above the cost model's
# estimate — keep everything on DVE)
GR = 8  # rows batched per output DMA

_CACHED = {}


def _emit(nc, tc, tile, mybir, tok_dram, out_dram, dyn_reps=None):
    alu = mybir.AluOpType

    with (
        tc.tile_pool(name="const", bufs=1) as const,
        tc.tile_pool(name="prep", bufs=1) as prep,
        tc.tile_pool(name="oh_a", bufs=24) as oh_a,
        tc.tile_pool(name="oh_b", bufs=24) as oh_b,
        tc.tile_pool(name="evac", bufs=3) as evac,
        tc.tile_pool(name="psum", bufs=8, space="PSUM") as psum,
    ):
        # constant iota rows (same in every partition)
        iota_i16 = const.tile([P, R2], mybir.dt.int16)
        nc.gpsimd.iota(iota_i16[:], pattern=[[1, R2]], base=0, channel_multiplier=0)
        iota_lo = const.tile([P, R2], mybir.dt.float16)
        nc.vector.tensor_copy(iota_lo[:], iota_i16[:])
        iota_hi = const.tile([P, R1], mybir.dt.float16)
        nc.vector.tensor_copy(iota_hi[:], iota_i16[:, :R1])

        # load tokens, shift by -1, split digits
        tok = prep.tile([P, S], mybir.dt.int32)
        nc.sync.dma_start(tok[:], tok_dram[:])
        u32 = prep.tile([P, S], mybir.dt.int32)
        nc.vector.tensor_scalar(u32[:], tok[:], 1, None, op0=alu.subtract)
        hprod = prep.tile([P, S], mybir.dt.int32)
        nc.vector.tensor_scalar(hprod[:], u32[:], DIV_MUL, None, op0=alu.mult)
        hi32 = prep.tile([P, S], mybir.dt.int32)
        nc.vector.tensor_scalar(
            hi32[:], hprod[:], DIV_SH, None, op0=alu.arith_shift_right
        )
        him = prep.tile([P, S], mybir.dt.int32)
        nc.vector.tensor_scalar(him[:], hi32[:], R2, None, op0=alu.mult)
        lo32 = prep.tile([P, S], mybir.dt.int32)
        nc.vector.tensor_tensor(lo32[:], u32[:], him[:], op=alu.subtract)

        hi16 = prep.tile([P, S], mybir.dt.int16)
        nc.vector.tensor_copy(hi16[:], hi32[:])
        lo16 = prep.tile([P, S], mybir.dt.int16)
        nc.vector.tensor_copy(lo16[:], lo32[:])

        # transpose each 128-col chunk: digT[p, c*128+b] = dig[b, c*128+p]
        hiT16 = prep.tile([P, S], mybir.dt.int16)
        loT16 = prep.tile([P, S], mybir.dt.int16)
        for c in range(NCH):
            sl = slice(c * P, (c + 1) * P)
            nc.sync.dma_start_transpose(hiT16[:, sl], hi16[:, sl])
            nc.sync.dma_start_transpose(loT16[:, sl], lo16[:, sl])

        # f32 per-partition scalar sources
        hiT = prep.tile([P, S], mybir.dt.float32)
        nc.vector.tensor_copy(hiT[:], hiT16[:])
        loT = prep.tile([P, S], mybir.dt.float32)
        nc.vector.tensor_copy(loT[:], loT16[:])

        def block_body(b0):
            # ev packs GR rows: ev[p, g*R2 + lo] = hist(row b0+g)[p*R2 + lo]
            ev = evac.tile([P, GR * R2], mybir.dt.float32)
            for g in range(GR):
                b = b0 + g
                ps = psum.tile([P, R2], mybir.dt.float32)
                for c in range(NCH):
                    col = c * P + b
                    b_t = oh_b.tile([P, R2], mybir.dt.float16)
                    nc.vector.tensor_scalar(
                        b_t[:], iota_lo[:], loT[:, col : col + 1], None,
                        op0=alu.is_equal,
                    )
                    a_t = oh_a.tile([P, R1], mybir.dt.float16)
                    eng = nc.gpsimd if c in POOL_CHUNKS else nc.vector
                    eng.tensor_scalar(
                        a_t[:], iota_hi[:], hiT[:, col : col + 1], None,
                        op0=alu.is_equal,
                    )
                    nc.tensor.matmul(
                        ps[:], a_t[:], b_t[:], start=(c == 0), stop=(c == NCH - 1)
                    )
                # DMA cannot read PSUM; evacuate via otherwise-idle ScalarE
                nc.scalar.copy(ev[:, g * R2 : (g + 1) * R2], ps[:])
            # v = 0..50037 for all GR rows in one DMA: DRAM traversal
            # (p, g, lo) at address (b0+g)*OUT_COLS + p*R2 + lo matches the
            # SBUF traversal (partition p, then free (g, lo)).
            big_dst = out_dram[b0 : b0 + GR, 0:BIG].rearrange(
                "g (p l) -> p g l", p=127, l=R2
            )
            nc.sync.dma_start(big_dst, ev[0:127, :])
            # v = 50038..50255 (partition 127, first TAIL cols of each row)
            tail_src = ev[127:128, :].rearrange("p (g l) -> p g l", g=GR, l=R2)[
                :, :, 0:TAIL
            ]
            nc.sync.dma_start(out_dram[b0 : b0 + GR, BIG:OUT_COLS], tail_src)

        if dyn_reps is None:
            for b0 in range(0, P, GR):
                block_body(b0)
        else:
            with tc.For_i(0, dyn_reps, 1):
                for b0 in range(0, P, GR):
                    block_body(b0)


def _build_program():
    import concourse.tile as tile
    from concourse import bacc, mybir

    nc = bacc.Bacc(
        "TRN2",
        target_bir_lowering=False,
        debug=False,
        enable_asserts=False,
        num_devices=N_CORES,
    )
    tok_dram = nc.dram_tensor("inputs", [P, S], mybir.dt.int32, kind="ExternalInput").ap()
    out_dram = nc.dram_tensor(
        "out", [P, OUT_COLS], mybir.dt.float32, kind="ExternalOutput"
    ).ap()

    with tile.TileContext(nc) as tc:
        _emit(nc, tc, tile, mybir, tok_dram, out_dram)

    nc.compile()
    return nc


def _build_program_dyn():
    """Variant with a runtime repeat loop around the row loop, for HW timing.

    Trip count comes from the extra [1,1] uint32 input "reps" — same NEFF for
    any R, so wall-time slope over R isolates device execution time.
    """
    import concourse.tile as tile
    from concourse import bacc, mybir

    nc = bacc.Bacc(
        "TRN2",
        target_bir_lowering=False,
        debug=False,
        enable_asserts=False,
        num_devices=N_CORES,
    )
    tok_dram = nc.dram_tensor("inputs", [P, S], mybir.dt.int32, kind="ExternalInput").ap()
    reps_dram = nc.dram_tensor("reps", [1, 1], mybir.dt.uint32, kind="ExternalInput").ap()
    out_dram = nc.dram_tensor(
        "out", [P, OUT_COLS], mybir.dt.float32, kind="ExternalOutput"
    ).ap()

    with tile.TileContext(nc) as tc:
        with tc.tile_pool(name="repsld", bufs=1) as repsld:
            reps_sb = repsld.tile([1, 1], mybir.dt.uint32)
            nc.sync.dma_start(reps_sb[:], reps_dram[:])

            from concourse.bass_primitives_rust import RegisterHandles
            from concourse.expressions import make_scalar_value

            regs = []
            for eng in (nc.sync, nc.vector, nc.scalar, nc.tensor, nc.gpsimd):
                tmp = eng.alloc_register(f"reps_{eng.engine.value}")
                eng.reg_load(tmp, reps_sb[0:1, 0:1])
                regs.append(tmp)
            rv = make_scalar_value(RegisterHandles(regs), min_val=0, max_val=1 << 20)

            _emit(nc, tc, tile, mybir, tok_dram, out_dram, dyn_reps=rv)

    nc.compile()
    return nc


def _get_program():
    if "nc" not in _CACHED:
        _CACHED["nc"] = _build_program()
    return _CACHED["nc"]


def kernel(inputs: np.ndarray, _trace: bool = False, _tmpdir: str | None = None):
    from concourse.bass_utils import run_bass_kernel_spmd

    nc = _get_program()
    inputs = np.ascontiguousarray(np.asarray(inputs, dtype=np.int32))
    assert inputs.shape == (B_FULL, S), inputs.shape
    in_maps = [{"inputs": inputs[k * P : (k + 1) * P]} for k in range(N_CORES)]
    res = run_bass_kernel_spmd(
        nc,
        in_maps,
        core_ids=list(range(N_CORES)),
        trace=_trace,
        tmpdir=_tmpdir,
    )
    out = np.concatenate([r["out"] for r in res.results], axis=0)
    if _trace:
        _CACHED["last_results"] = res
    return out


# revision 8
# speedup vs baseline: 1.1822x; 1.0407x over previous
"""Bag-of-words histogram kernel for Trainium2 (8 NeuronCores, data-parallel).

Problem: inputs [1024, 512] int32 token ids in [0, 50257); output [1024, 50256]
f32 per-row token-count histogram with token id 0 dropped.

Design (per core, 128 rows):
  Shifted token u = t - 1 decomposes as u = hi*394 + lo with hi in [-1,128),
  lo in [0,394) (exact integer div via multiply-arith-shift: hi =
  (u*21291)>>23, verified offline for all u in [-1, 50256)). Token t=0 gives
  hi=-1 which matches no hi-bin, so the "drop token 0" semantics falls out of
  the decomposition for free and output bin v = u = hi*394 + lo directly.

  Per row the histogram is hist[hi, lo] = sum_j onehot_hi(u_j)[hi] *
  onehot_lo(u_j)[lo]: a matmul with lhsT = A [K=128 tokens, M=128 hi-bins]
  f16 one-hot, rhs = B [K=128 tokens, N=394 lo-bins] f16 one-hot, accumulated
  over 4 K-chunks of 128 tokens into one PSUM bank [128, 394] f32. Duplicate
  tokens are handled exactly by the accumulation.

  Engine split (from CoreSim engine-busy analysis):
    - DVE builds all wide B one-hots (f16, 4x perf mode).
    - GPSIMD (otherwise idle) builds all A one-hots.
    - ACT only evacuates PSUM -> SBUF f32, packing GR=8 rows per wide tile.
    - SP issues the output DMAs. Every HWDGE dma_start costs ~625ns of the
      shared descriptor generator regardless of size, so DMAs are batched:
      one [127-partition x 8-row x 1576B-line] piece (v = 0..50037 for 8
      rows) plus one batched tail piece (v = 50038..50255, partition 127)
      per 8 rows -> 32 output DMAs total instead of 384.

  One-hots are built with tensor_scalar(is_equal) against constant iota rows,
  using per-partition f32 scalars = the transposed hi/lo digit of each token
  (DMA-transposed i16 digit tensors).
"""

import sys

sys.path.insert(0, "/opt/trn_rl_repo")

import numpy as np

N_CORES = 8
B_FULL = 1024
P = 128  # rows per core / partitions
S = 512  # tokens per row
V = 50257
R1 = 128  # hi bins (partition dim of PSUM)
R2 = 394  # lo bins (free dim of PSUM)
DIV_MUL = 21291  # (u*DIV_MUL)>>23 (arith) == u//394 for all u in [-1, 50256)
DIV_SH = 23
NCH = S // P  # 4 K-chunks per row
OUT_COLS = V - 1  # 50256

BIG = 127 * R2  # 50038: v = 0..50037 from partitions 0..126
TAIL = OUT_COLS - BIG  # 218: v = 50038..50255 from partition 127 cols 0..217

POOL_CHUNKS = ()  # a_t one-hots built on GPSIMD for these chunks (HW: GPSIMD
# tensor ops cost ~2us launch overhead each, far above the cost model's
# estimate — keep everything on DVE)
GR = 8  # rows batched per output DMA

_CACHED = {}


def _emit(nc, tc, tile, mybir, tok_dram, out_dram, dyn_reps=None):
    alu = mybir.AluOpType

    with (
        tc.tile_pool(name="const", bufs=1) as const,
        tc.tile_pool(name="prep", bufs=1) as prep,
        tc.tile_pool(name="oh_a", bufs=24) as oh_a,
        tc.tile_pool(name="oh_b", bufs=24) as oh_b,
        tc.tile_pool(name="evac", bufs=3) as evac,
        tc.tile_pool(name="psum", bufs=8, space="PSUM") as psum,
    ):
        # constant iota rows (same in every partition)
        iota_i16 = const.tile([P, R2], mybir.dt.int16)
        nc.gpsimd.iota(iota_i16[:], pattern=[[1, R2]], base=0, channel_multiplier=0)
        iota_lo = const.tile([P, R2], mybir.dt.float16)
        nc.vector.tensor_copy(iota_lo[:], iota_i16[:])
        iota_hi = const.tile([P, R1], mybir.dt.float16)
        nc.vector.tensor_copy(iota_hi[:], iota_i16[:, :R1])

        # load tokens, shift by -1, split digits
        tok = prep.tile([P, S], mybir.dt.int32)
        nc.sync.dma_start(tok[:], tok_dram[:])
        u32 = prep.tile([P, S], mybir.dt.int32)
        nc.vector.tensor_scalar(u32[:], tok[:], 1, None, op0=alu.subtract)
        hprod = prep.tile([P, S], mybir.dt.int32)
        nc.vector.tensor_scalar(hprod[:], u32[:], DIV_MUL, None, op0=alu.mult)
        hi32 = prep.tile([P, S], mybir.dt.int32)
        nc.vector.tensor_scalar(
            hi32[:], hprod[:], DIV_SH, None, op0=alu.arith_shift_right
        )
        him = prep.tile([P, S], mybir.dt.int32)
        nc.vector.tensor_scalar(him[:], hi32[:], R2, None, op0=alu.mult)
        lo32 = prep.tile([P, S], mybir.dt.int32)
        nc.vector.tensor_tensor(lo32[:], u32[:], him[:], op=alu.subtract)

        hi16 = prep.tile([P, S], mybir.dt.int16)
        nc.vector.tensor_copy(hi16[:], hi32[:])
        lo16 = prep.tile([P, S], mybir.dt.int16)
        nc.vector.tensor_copy(lo16[:], lo32[:])

        # transpose each 128-col chunk: digT[p, c*128+b] = dig[b, c*128+p]
        hiT16 = prep.tile([P, S], mybir.dt.int16)
        loT16 = prep.tile([P, S], mybir.dt.int16)
        for c in range(NCH):
            sl = slice(c * P, (c + 1) * P)
            nc.sync.dma_start_transpose(hiT16[:, sl], hi16[:, sl])
            nc.sync.dma_start_transpose(loT16[:, sl], lo16[:, sl])

        # f32 per-partition scalar sources
        hiT = prep.tile([P, S], mybir.dt.float32)
        nc.vector.tensor_copy(hiT[:], hiT16[:])
        loT = prep.tile([P, S], mybir.dt.float32)
        nc.vector.tensor_copy(loT[:], loT16[:])

        def block_body(b0):
            # ev packs GR rows: ev[p, g*R2 + lo] = hist(row b0+g)[p*R2 + lo]
            ev = evac.tile([P, GR * R2], mybir.dt.float32)
            for g in range(GR):
                b = b0 + g
                ps = psum.tile([P, R2], mybir.dt.float32)
                for c in range(NCH):
                    col = c * P + b
                    b_t = oh_b.tile([P, R2], mybir.dt.float16)
                    nc.vector.tensor_scalar(
                        b_t[:], iota_lo[:], loT[:, col : col + 1], None,
                        op0=alu.is_equal,
                    )
                    a_t = oh_a.tile([P, R1], mybir.dt.float16)
                    eng = nc.gpsimd if c in POOL_CHUNKS else nc.vector
                    eng.tensor_scalar(
                        a_t[:], iota_hi[:], hiT[:, col : col + 1], None,
                        op0=alu.is_equal,
                    )
                    nc.tensor.matmul(
                        ps[:], a_t[:], b_t[:], start=(c == 0), stop=(c == NCH - 1)
                    )
                # DMA cannot read PSUM; evacuate via otherwise-idle ScalarE
                nc.scalar.copy(ev[:, g * R2 : (g + 1) * R2], ps[:])
                # v = 0..50037: per-row 2D DMA (127 partition lines of 1576B —
                # the PDMA2D-friendly shape; 3D batched APs fall off the HW
                # descriptor fast path)
                nc.sync.dma_start(
                    out_dram[b, 0:BIG], ev[0:127, g * R2 : (g + 1) * R2]
                )
            # v = 50038..50255 for all GR rows in one 2D DMA (partition 127,
            # first TAIL cols of each row's segment)
            tail_src = ev[127:128, :].rearrange("p (g l) -> p g l", g=GR, l=R2)[
                :, :, 0:TAIL
            ]
            nc.scalar.dma_start(out_dram[b0 : b0 + GR, BIG:OUT_COLS], tail_src)

        if dyn_reps is None:
            for b0 in range(0, P, GR):
                block_body(b0)
        else:
            with tc.For_i(0, dyn_reps, 1):
                for b0 in range(0, P, GR):
                    block_body(b0)


def _build_program():
    import concourse.tile as tile
    from concourse import bacc, mybir

    nc = bacc.Bacc(
        "TRN2",
        target_bir_lowering=False,
        debug=False,
        enable_asserts=False,
        num_devices=N_CORES,
    )
    tok_dram = nc.dram_tensor("inputs", [P, S], mybir.dt.int32, kind="ExternalInput").ap()
    out_dram = nc.dram_tensor(
        "out", [P, OUT_COLS], mybir.dt.float32, kind="ExternalOutput"
    ).ap()

    with tile.TileContext(nc) as tc:
        _emit(nc, tc, tile, mybir, tok_dram, out_dram)

    nc.compile()
    return nc


def _build_program_dyn():
    """Variant with a runtime repeat loop around the row loop, for HW timing.

    Trip count comes from the extra [1,1] uint32 input "reps" — same NEFF for
    any R, so wall-time slope over R isolates device execution time.
    """
    import concourse.tile as tile
    from concourse import bacc, mybir

    nc = bacc.Bacc(
        "TRN2",
        target_bir_lowering=False,
        debug=False,
        enable_asserts=False,
        num_devices=N_CORES,
    )
    tok_dram = nc.dram_tensor("inputs", [P, S], mybir.dt.int32, kind="ExternalInput").ap()
    reps_dram = nc.dram_tensor("reps", [1, 1], mybir.dt.uint32, kind="ExternalInput").ap()
    out_dram = nc.dram_tensor(
        "out", [P, OUT_COLS], mybir.dt.float32, kind="ExternalOutput"
    ).ap()

    with tile.TileContext(nc) as tc:
        with tc.tile_pool(name="repsld", bufs=1) as repsld:
            reps_sb = repsld.tile([1, 1], mybir.dt.uint32)
            nc.sync.dma_start(reps_sb[:], reps_dram[:])

            from concourse.bass_primitives_rust import RegisterHandles
            from concourse.expressions import make_scalar_value

            regs = []
            for eng in (nc.sync, nc.vector, nc.scalar, nc.tensor, nc.gpsimd):
                tmp = eng.alloc_register(f"reps_{eng.engine.value}")
                eng.reg_load(tmp, reps_sb[0:1, 0:1])
                regs.append(tmp)
            rv = make_scalar_value(RegisterHandles(regs), min_val=0, max_val=1 << 20)

            _emit(nc, tc, tile, mybir, tok_dram, out_dram, dyn_reps=rv)

    nc.compile()
    return nc


def _get_program():
    if "nc" not in _CACHED:
        _CACHED["nc"] = _build_program()
    return _CACHED["nc"]


def kernel(inputs: np.ndarray, _trace: bool = False, _tmpdir: str | None = None):
    from concourse.bass_utils import run_bass_kernel_spmd

    nc = _get_program()
    inputs = np.ascontiguousarray(np.asarray(inputs, dtype=np.int32))
    assert inputs.shape == (B_FULL, S), inputs.shape
    in_maps = [{"inputs": inputs[k * P : (k + 1) * P]} for k in range(N_CORES)]
    res = run_bass_kernel_spmd(
        nc,
        in_maps,
        core_ids=list(range(N_CORES)),
        trace=_trace,
        tmpdir=_tmpdir,
    )
    out = np.concatenate([r["out"] for r in res.results], axis=0)
    if _trace:
        _CACHED["last_results"] = res
    return out


# revision 9
# speedup vs baseline: 9.3442x; 7.9044x over previous
"""Bag-of-words histogram kernel for Trainium2 (8 NeuronCores, data-parallel).

Problem: inputs [1024, 512] int32 token ids in [0, 50257); output [1024, 50256]
f32 per-row token-count histogram with token id 0 dropped.

Design (per core, 128 rows):
  Shifted token u = t - 1 decomposes as u = hi*394 + lo with hi in [-1,128),
  lo in [0,394) (exact integer div via multiply-arith-shift: hi =
  (u*21291)>>23, verified offline for all u in [-1, 50256)). Token t=0 gives
  hi=-1 which matches no hi-bin, so the "drop token 0" semantics falls out of
  the decomposition for free and output bin v = u = hi*394 + lo directly.

  Per row the histogram is hist[hi, lo] = sum_j onehot_hi(u_j)[hi] *
  onehot_lo(u_j)[lo]: a matmul with lhsT = A [K=128 tokens, M=128 hi-bins]
  f16 one-hot, rhs = B [K=128 tokens, N=394 lo-bins] f16 one-hot, accumulated
  over 4 K-chunks of 128 tokens into one PSUM bank [128, 394] f32. Duplicate
  tokens are handled exactly by the accumulation.

  Engine split (from CoreSim engine-busy analysis):
    - DVE builds all wide B one-hots (f16, 4x perf mode).
    - GPSIMD (otherwise idle) builds all A one-hots.
    - ACT only evacuates PSUM -> SBUF f32, packing GR=8 rows per wide tile.
    - SP issues the output DMAs. Every HWDGE dma_start costs ~625ns of the
      shared descriptor generator regardless of size, so DMAs are batched:
      one [127-partition x 8-row x 1576B-line] piece (v = 0..50037 for 8
      rows) plus one batched tail piece (v = 50038..50255, partition 127)
      per 8 rows -> 32 output DMAs total instead of 384.

  One-hots are built with tensor_scalar(is_equal) against constant iota rows,
  using per-partition f32 scalars = the transposed hi/lo digit of each token
  (DMA-transposed i16 digit tensors).
"""

import sys

sys.path.insert(0, "/opt/trn_rl_repo")

import numpy as np

N_CORES = 8
B_FULL = 1024
P = 128  # rows per core / partitions
S = 512  # tokens per row
V = 50257
R1 = 128  # hi bins (partition dim of PSUM)
R2 = 394  # lo bins (free dim of PSUM)
DIV_MUL = 21291  # (u*DIV_MUL)>>23 (arith) == u//394 for all u in [-1, 50256)
DIV_SH = 23
NCH = S // P  # 4 K-chunks per row
OUT_COLS = V - 1  # 50256

BIG = 127 * R2  # 50038: v = 0..50037 from partitions 0..126
TAIL = OUT_COLS - BIG  # 218: v = 50038..50255 from partition 127 cols 0..217

POOL_CHUNKS = ()  # a_t one-hots built on GPSIMD for these chunks (HW: GPSIMD
# tensor ops cost ~2us launch overhead each, far above the cost model's
# estimate — keep everything on DVE)
GR = 8  # rows batched per output DMA

_CACHED = {}


def _emit(nc, tc, tile, mybir, tok_dram, out_dram, dyn_reps=None):
    alu = mybir.AluOpType

    with (
        tc.tile_pool(name="const", bufs=1) as const,
        tc.tile_pool(name="prep", bufs=1) as prep,
        tc.tile_pool(name="oh_a", bufs=24) as oh_a,
        tc.tile_pool(name="oh_b", bufs=24) as oh_b,
        tc.tile_pool(name="evac", bufs=3) as evac,
        tc.tile_pool(name="psum", bufs=8, space="PSUM") as psum,
    ):
        # constant iota rows (same in every partition)
        iota_i16 = const.tile([P, R2], mybir.dt.int16)
        nc.gpsimd.iota(iota_i16[:], pattern=[[1, R2]], base=0, channel_multiplier=0)
        iota_lo = const.tile([P, R2], mybir.dt.float16)
        nc.vector.tensor_copy(iota_lo[:], iota_i16[:])
        iota_hi = const.tile([P, R1], mybir.dt.float16)
        nc.vector.tensor_copy(iota_hi[:], iota_i16[:, :R1])

        # load tokens, shift by -1, split digits
        tok = prep.tile([P, S], mybir.dt.int32)
        nc.sync.dma_start(tok[:], tok_dram[:])
        u32 = prep.tile([P, S], mybir.dt.int32)
        nc.vector.tensor_scalar(u32[:], tok[:], 1, None, op0=alu.subtract)
        hprod = prep.tile([P, S], mybir.dt.int32)
        nc.vector.tensor_scalar(hprod[:], u32[:], DIV_MUL, None, op0=alu.mult)
        hi32 = prep.tile([P, S], mybir.dt.int32)
        nc.vector.tensor_scalar(
            hi32[:], hprod[:], DIV_SH, None, op0=alu.arith_shift_right
        )
        him = prep.tile([P, S], mybir.dt.int32)
        nc.vector.tensor_scalar(him[:], hi32[:], R2, None, op0=alu.mult)
        lo32 = prep.tile([P, S], mybir.dt.int32)
        nc.vector.tensor_tensor(lo32[:], u32[:], him[:], op=alu.subtract)

        hi16 = prep.tile([P, S], mybir.dt.int16)
        nc.vector.tensor_copy(hi16[:], hi32[:])
        lo16 = prep.tile([P, S], mybir.dt.int16)
        nc.vector.tensor_copy(lo16[:], lo32[:])

        # transpose each 128-col chunk: digT[p, c*128+b] = dig[b, c*128+p]
        hiT16 = prep.tile([P, S], mybir.dt.int16)
        loT16 = prep.tile([P, S], mybir.dt.int16)
        for c in range(NCH):
            sl = slice(c * P, (c + 1) * P)
            nc.sync.dma_start_transpose(hiT16[:, sl], hi16[:, sl])
            nc.sync.dma_start_transpose(loT16[:, sl], lo16[:, sl])

        # f32 per-partition scalar sources
        hiT = prep.tile([P, S], mybir.dt.float32)
        nc.vector.tensor_copy(hiT[:], hiT16[:])
        loT = prep.tile([P, S], mybir.dt.float32)
        nc.vector.tensor_copy(loT[:], loT16[:])

        def block_body(b0):
            # ev packs GR rows: ev[p, g*R2 + lo] = hist(row b0+g)[p*R2 + lo]
            ev = evac.tile([P, GR * R2], mybir.dt.float32)
            for g in range(GR):
                b = b0 + g
                ps = psum.tile([P, R2], mybir.dt.float32)
                for c in range(NCH):
                    col = c * P + b
                    b_t = oh_b.tile([P, R2], mybir.dt.float16)
                    nc.vector.tensor_scalar(
                        b_t[:], iota_lo[:], loT[:, col : col + 1], None,
                        op0=alu.is_equal,
                    )
                    a_t = oh_a.tile([P, R1], mybir.dt.float16)
                    eng = nc.gpsimd if c in POOL_CHUNKS else nc.vector
                    eng.tensor_scalar(
                        a_t[:], iota_hi[:], hiT[:, col : col + 1], None,
                        op0=alu.is_equal,
                    )
                    nc.tensor.matmul(
                        ps[:], a_t[:], b_t[:], start=(c == 0), stop=(c == NCH - 1)
                    )
                # DMA cannot read PSUM; evacuate via otherwise-idle ScalarE
                nc.scalar.copy(ev[:, g * R2 : (g + 1) * R2], ps[:])
                # v = 0..50037: per-row 2D DMA (127 partition lines of 1576B —
                # the PDMA2D-friendly shape; 3D batched APs fall off the HW
                # descriptor fast path)
                nc.sync.dma_start(
                    out_dram[b, 0:BIG], ev[0:127, g * R2 : (g + 1) * R2]
                )
                # v = 50038..50255: per-row contiguous tail (partition 127)
                nc.scalar.dma_start(
                    out_dram[b, BIG:OUT_COLS],
                    ev[127:128, g * R2 : g * R2 + TAIL],
                )

        if dyn_reps is None:
            for b0 in range(0, P, GR):
                block_body(b0)
        else:
            with tc.For_i(0, dyn_reps, 1):
                for b0 in range(0, P, GR):
                    block_body(b0)


def _build_program():
    import concourse.tile as tile
    from concourse import bacc, mybir

    nc = bacc.Bacc(
        "TRN2",
        target_bir_lowering=False,
        debug=False,
        enable_asserts=False,
        num_devices=N_CORES,
    )
    tok_dram = nc.dram_tensor("inputs", [P, S], mybir.dt.int32, kind="ExternalInput").ap()
    out_dram = nc.dram_tensor(
        "out", [P, OUT_COLS], mybir.dt.float32, kind="ExternalOutput"
    ).ap()

    with tile.TileContext(nc) as tc:
        _emit(nc, tc, tile, mybir, tok_dram, out_dram)

    nc.compile()
    return nc


def _build_program_dyn():
    """Variant with a runtime repeat loop around the row loop, for HW timing.

    Trip count comes from the extra [1,1] uint32 input "reps" — same NEFF for
    any R, so wall-time slope over R isolates device execution time.
    """
    import concourse.tile as tile
    from concourse import bacc, mybir

    nc = bacc.Bacc(
        "TRN2",
        target_bir_lowering=False,
        debug=False,
        enable_asserts=False,
        num_devices=N_CORES,
    )
    tok_dram = nc.dram_tensor("inputs", [P, S], mybir.dt.int32, kind="ExternalInput").ap()
    reps_dram = nc.dram_tensor("reps", [1, 1], mybir.dt.uint32, kind="ExternalInput").ap()
    out_dram = nc.dram_tensor(
        "out", [P, OUT_COLS], mybir.dt.float32, kind="ExternalOutput"
    ).ap()

    with tile.TileContext(nc) as tc:
        with tc.tile_pool(name="repsld", bufs=1) as repsld:
            reps_sb = repsld.tile([1, 1], mybir.dt.uint32)
            nc.sync.dma_start(reps_sb[:], reps_dram[:])

            from concourse.bass_primitives_rust import RegisterHandles
            from concourse.expressions import make_scalar_value

            regs = []
            for eng in (nc.sync, nc.vector, nc.scalar, nc.tensor, nc.gpsimd):
                tmp = eng.alloc_register(f"reps_{eng.engine.value}")
                eng.reg_load(tmp, reps_sb[0:1, 0:1])
                regs.append(tmp)
            rv = make_scalar_value(RegisterHandles(regs), min_val=0, max_val=1 << 20)

            _emit(nc, tc, tile, mybir, tok_dram, out_dram, dyn_reps=rv)

    nc.compile()
    return nc


def _get_program():
    if "nc" not in _CACHED:
        _CACHED["nc"] = _build_program()
    return _CACHED["nc"]


def kernel(inputs: np.ndarray, _trace: bool = False, _tmpdir: str | None = None):
    from concourse.bass_utils import run_bass_kernel_spmd

    nc = _get_program()
    inputs = np.ascontiguousarray(np.asarray(inputs, dtype=np.int32))
    assert inputs.shape == (B_FULL, S), inputs.shape
    in_maps = [{"inputs": inputs[k * P : (k + 1) * P]} for k in range(N_CORES)]
    res = run_bass_kernel_spmd(
        nc,
        in_maps,
        core_ids=list(range(N_CORES)),
        trace=_trace,
        tmpdir=_tmpdir,
    )
    out = np.concatenate([r["out"] for r in res.results], axis=0)
    if _trace:
        _CACHED["last_results"] = res
    return out
